# revision 17
# baseline (speedup 1.0000x reference)
"""ContrastiveKernelLoss on 8 Trainium2 cores.

Math: for each layer, D[i,j] = ||I - inv(kn_i) @ kn_j||_F over the n x n pair
grid.  ||I - A B||_F^2 = d - 2*tr(A B) + ||A B||_F^2 expands into dot products
over per-kernel features, so the whole grid is one skinny matmul:
  X = W^T R,  W rows = [-2*vec(inv_i); vec(inv_i^T inv_i); d; 1],
              R rows = [vec(kn_j^T);  vec(kn_j kn_j^T);   1; m*_j]
The loss needs only the strict lower triangle (combined = tril(D)+triu(D^T)).
Each core computes a balanced subset of 128x512 lower-triangle tiles
("slots").  Masking of j >= i positions pushes X up by BIG so the hinge is 0:
  - column-only mask regions ride the extra rank-1 feature row (m*, staged
    per core — the program stays SPMD-uniform),
  - the 128x128 diagonal block is column-permuted to the tail of its chunk,
    where one shared strict-upper triangle constant is added via a second
    matmul (identity lhsT) — identical addresses on every core.
Per slot: PE matmul(s) -> ACT sqrt -> DVE min(s,1) with fused row-sum.
sum(relu(1-D)) = n_cols - sum(min(sqrt(X),1)) exactly, because inactive
entries contribute exact 1.0.  Host reduces in float64.
"""

import numpy as np
from contextlib import ExitStack

EPS = 1e-8
# Added to masked grid entries before sqrt.  Must dominate the matmul
# rounding error of the diagonal cancellation (can reach ~1e5 for
# near-singular kernels); 2^26 is exactly representable in fp32r.
BIG = float(2 ** 26)
N0, D0 = 2048, 3
N1, D1 = 1024, 5
F0 = 2 * D0 * D0 + 2   # 20: features + bias row + mask row
F1 = 2 * D1 * D1 + 2   # 52
NSLOT = 7              # 5 layer0 slots + 2 layer1 slots per core
N_CORES = 8

TRACE = False          # test harness sets True to capture an NTFF profile
LAST_RESULT = None     # BassKernelResults of the most recent run

_cached_nc = None


def _rtne12(x):
    """Round f32 to fp32r (11 explicit mantissa bits, round-nearest-even) —
    bit-exact match for what the PE consumes (verified on HW)."""
    u = np.ascontiguousarray(x, np.float32).view(np.uint32).astype(np.uint64)
    add = np.uint64(0x7FF) + ((u >> np.uint64(12)) & np.uint64(1))
    return ((u + add) >> np.uint64(12) << np.uint64(12)).astype(
        np.uint32).view(np.float32)


def _features(kernels, d):
    """Per-kernel features in float64: W (F, n), R (F, n).

    Last row of R (the mask row) is zero here; _stage_core fills it per slot.
    """
    k = np.asarray(kernels, np.float64)
    n = k.shape[0]
    norms = np.sqrt((k * k).sum(axis=(1, 2), keepdims=True)) + EPS
    kn = k / norms
    inv = np.linalg.inv(kn)
    G = np.einsum('iba,ibc->iac', inv, inv)        # inv^T @ inv
    H = np.einsum('iab,icb->iac', kn, kn)          # kn @ kn^T
    W = np.concatenate([
        -2.0 * inv.reshape(n, -1),
        G.reshape(n, -1),
        np.full((n, 1), float(d)),
        np.ones((n, 1)),
    ], axis=1).T
    R = np.concatenate([
        np.transpose(kn, (0, 2, 1)).reshape(n, -1),
        H.reshape(n, -1),
        np.ones((n, 1)),
        np.zeros((n, 1)),
    ], axis=1).T
    return W, R


def _diag_slot(W, R, t):
    """Build (w, r) for a diagonal slot of row tile t: columns permuted to
    [left | right | block] with the mask row set to BIG on the right region.
    The shared triangle constant later masks n' >= m inside the block."""
    F = W.shape[0]
    c = t // 4
    L = 128 * (t % 4)
    r = np.empty((F, 512), np.float64)
    r[:, :L] = R[:, 512 * c:512 * c + L]                    # left: keep
    r[:, L:384] = R[:, 128 * t + 128:512 * (c + 1)]         # right: mask
    r[:, 384:] = R[:, 128 * t:128 * t + 128]                # diagonal block
    r[F - 1, :] = 0.0
    r[F - 1, L:384] = BIG
    return W[:, 128 * t:128 * (t + 1)], r


def _full_slot(W, R, t, c):
    return (W[:, 128 * t:128 * (t + 1)],
            np.ascontiguousarray(R[:, 512 * c:512 * (c + 1)]))


def _core_slots(core, W0, R0, W1, R1):
    """7 slots: list of (w [F,128], r [F,512], has_corr).  Slots 0,1 are
    layer0 diagonal tiles, 2-4 layer0 full chunks, 5 layer1 diagonal,
    6 layer1 full chunk (cores >= 4) or fully-masked dummy (cores < 4)."""
    tA, tB = (core, 12 + core) if core < 4 else (core, core + 4)
    slots = [(*_diag_slot(W0, R0, t), True) for t in (tA, tB)]
    fulls = [(tA, c) for c in range(tA // 4)] + [(tB, c) for c in range(tB // 4)]
    assert len(fulls) == 3
    slots += [(*_full_slot(W0, R0, t, c), False) for (t, c) in fulls]
    t = core
    slots.append((*_diag_slot(W1, R1, t), True))
    w6, r6 = _full_slot(W1, R1, t, 0)
    if core < 4:
        r6 = r6.copy()
        r6[F1 - 1, :] = BIG                     # dummy: fully masked
    slots.append((w6, r6, False))
    return slots


def _split(a):
    """fp32r hi/lo split: a ~= hi + lo with both fp32r-exact."""
    hi = _rtne12(a.astype(np.float32))
    lo = _rtne12((a - hi.astype(np.float64)).astype(np.float32))
    return hi, lo


def _stage_core(core, W0, R0, W1, R1):
    """Per-core input map.

    Layer0 slots are error-compensated via one K=3*F0 matmul: wr0[s] is
    [3*F0, 640] staged as lhsT rows [Wh; Wl; Wh] and rhs rows [Rh; Rh; Rl],
    so X = Wh.Rh + Wl.Rh + Wh.Rl — fp32r speed, ~f32 accuracy (only the
    ~2^-24 Wl.Rl term is dropped).  Layer1 is single-pass fp32r [F1, 640]:
    it has no active pairs and a ~200x margin to the hinge threshold, so
    fp32r noise is harmless.
    """
    slots = _core_slots(core, W0, R0, W1, R1)
    wr0 = np.empty((5, 3 * F0, 640), np.float32)
    wr1 = np.empty((2, F1, 640), np.float32)
    for s, (w, r, _) in enumerate(slots):
        if s < 5:
            wh, wl = _split(w)
            rh, rl = _split(r)
            wr0[s, :F0, :128] = wh
            wr0[s, :F0, 128:] = rh
            wr0[s, F0:2 * F0, :128] = wl
            wr0[s, F0:2 * F0, 128:] = rh
            wr0[s, 2 * F0:, :128] = wh
            wr0[s, 2 * F0:, 128:] = rl
        else:
            wr1[s - 5, :, :128] = _rtne12(w.astype(np.float32))
            wr1[s - 5, :, 128:] = _rtne12(r.astype(np.float32))
    return {"wr0": wr0, "wr1": wr1}


def _build_program():
    import concourse.bacc as bacc
    import concourse.tile as tile
    import concourse.mybir as mybir

    f32 = mybir.dt.float32
    f32r = mybir.dt.float32r
    nc = bacc.Bacc("TRN2")
    wr0 = nc.declare_dram_parameter("wr0", [5, 3 * F0, 640], f32r,
                                    isOutput=False)
    wr1 = nc.declare_dram_parameter("wr1", [2, F1, 640], f32r, isOutput=False)
    acc = nc.declare_dram_parameter("acc", [NSLOT, 1], f32, isOutput=True)

    with tile.TileContext(nc) as tc, ExitStack() as ctx:
        cpool = ctx.enter_context(tc.tile_pool(name="const", bufs=1))
        ppool = ctx.enter_context(
            tc.tile_pool(name="psum", bufs=4, space="PSUM"))
        spool = ctx.enter_context(tc.tile_pool(name="work", bufs=3))

        # Per-slot fused (w | r) tiles; separate DMAs, issued round-robin
        # across engine sequencers (each DMA_DIRECT2D occupies its issuing
        # sequencer ~0.8-1us) so issue and transfer both parallelize.
        dma_engines = [nc.sync, nc.scalar, nc.gpsimd]
        wr_tiles = []
        for s in range(NSLOT):
            F = 3 * F0 if s < 5 else F1
            src = wr0.ap()[s] if s < 5 else wr1.ap()[s - 5]
            t = cpool.tile([F, 640], f32r, tag=f"wr{s}")
            dma_engines[s % len(dma_engines)].dma_start(t[:], src)
            wr_tiles.append(t)

        # Shared constants, generated on-device: identity (corr lhsT) and
        # the strict-upper triangle TRI[k, n] = BIG * (n >= k).
        iota_c = cpool.tile([128, 128], f32)
        nc.gpsimd.iota(iota_c[:], pattern=[[1, 128]], base=0,
                       channel_multiplier=0,
                       allow_small_or_imprecise_dtypes=True)
        iota_p = cpool.tile([128, 1], f32)
        nc.gpsimd.iota(iota_p[:], pattern=[[1, 1]], base=0,
                       channel_multiplier=1,
                       allow_small_or_imprecise_dtypes=True)
        ident = cpool.tile([128, 128], f32r)
        nc.vector.tensor_scalar(
            out=ident[:], in0=iota_c[:], scalar1=iota_p[:, 0:1], scalar2=None,
            op0=mybir.AluOpType.is_equal)
        tri = cpool.tile([128, 128], f32r)
        nc.vector.tensor_scalar(
            out=tri[:], in0=iota_c[:], scalar1=iota_p[:, 0:1], scalar2=BIG,
            op0=mybir.AluOpType.is_ge, op1=mybir.AluOpType.mult)

        ones = cpool.tile([128, 1], f32)
        nc.gpsimd.memset(ones[:], 1.0)

        acc_t = cpool.tile([128, NSLOT], f32)
        has_corr = [True, True, False, False, False, True, False]
        for s in range(NSLOT):
            wr = wr_tiles[s]
            ps = ppool.tile([128, 512], f32)
            nc.tensor.matmul(ps[:], wr[:, 0:128], wr[:, 128:640],
                             start=True, stop=not has_corr[s])
            if has_corr[s]:
                nc.tensor.matmul(ps[:, 384:512], ident[:], tri[:],
                                 start=False, stop=True)
            sq = spool.tile([128, 512], f32, tag="sq")
            nc.scalar.activation(sq[:], ps[:],
                                 mybir.ActivationFunctionType.Sqrt)
            sc = spool.tile([128, 512], f32, tag="sc")
            nc.vector.tensor_scalar(
                out=sc[:], in0=sq[:], scalar1=1.0, scalar2=None,
                op0=mybir.AluOpType.min, op1=mybir.AluOpType.add,
                accum_out=acc_t[:, s:s + 1])
        # Partition-reduce acc on PE (out[s] = sum_p acc[p, s]) so the output
        # DMA is NSLOT packets instead of 128.
        # Subtract the 512-per-row baseline first: the remaining values are
        # exact zeros except rows with active pairs, so the f32 PE reduction
        # is exact to ~1e-7 (summing raw ~65536-magnitude rows costs ~1e-3).
        acc_d = cpool.tile([128, NSLOT], f32)
        nc.vector.tensor_scalar(
            out=acc_d[:], in0=acc_t[:], scalar1=512.0, scalar2=None,
            op0=mybir.AluOpType.subtract)
        acc_ps = ppool.tile([NSLOT, 1], f32, tag="accps")
        nc.tensor.matmul(acc_ps[:], acc_d[:], ones[:], start=True, stop=True)
        acc_sb = cpool.tile([NSLOT, 1], f32)
        nc.scalar.copy(acc_sb[:], acc_ps[:])
        nc.sync.dma_start(acc.ap(), acc_sb[:])
    nc.compile()
    return nc


def _emulate_acc(in_map):
    """Numpy emulation of the device program (for sim/HW debugging)."""
    acc = np.zeros((128, NSLOT), np.float32)
    tri = (np.arange(128)[None, :] >= np.arange(128)[:, None]) * np.float32(BIG)
    has_corr = [True, True, False, False, False, True, False]
    for s in range(NSLOT):
        wr = in_map["wr0"][s] if s < 5 else in_map["wr1"][s - 5]
        x = wr[:, :128].T.astype(np.float32) @ wr[:, 128:]
        if has_corr[s]:
            x[:, 384:] += tri
        acc[:, s] = np.minimum(np.sqrt(x), 1.0).sum(axis=1)
    return (acc - np.float32(512.0)).sum(axis=0).reshape(NSLOT, 1)


def _reduce_loss(accs):
    """Host-side float64 reduction of per-core [NSLOT, 1] accumulators.

    Device returns sum_p (acc[p, s] - 512), so each slot's hinge total is
    just the negation.
    """
    S0 = 0.0
    S1 = 0.0
    for a in accs:
        a = np.asarray(a, np.float64).reshape(NSLOT)
        S0 += -a[:5].sum()
        S1 += -a[5:].sum()
    loss = 0.5 * (2.0 * S0 / (N0 * (N0 - 1)) + 2.0 * S1 / (N1 * (N1 - 1)))
    return np.float32(loss)


def _get_program():
    global _cached_nc
    if _cached_nc is None:
        _cached_nc = _build_program()
    return _cached_nc


def kernel(kernels0, kernels1):
    global LAST_RESULT
    from concourse.bass_utils import run_bass_kernel_spmd

    W0, R0 = _features(kernels0, D0)
    W1, R1 = _features(kernels1, D1)
    in_maps = [_stage_core(c, W0, R0, W1, R1) for c in range(N_CORES)]
    nc = _get_program()
    res = run_bass_kernel_spmd(nc, in_maps, list(range(N_CORES)), trace=TRACE)
    LAST_RESULT = res
    accs = [res.results[c]["acc"] for c in range(N_CORES)]
    return _reduce_loss(accs)


# revision 19
# speedup vs baseline: 1.0226x; 1.0226x over previous
"""ContrastiveKernelLoss on 8 Trainium2 cores.

Math: for each layer, D[i,j] = ||I - inv(kn_i) @ kn_j||_F over the n x n pair
grid.  ||I - A B||_F^2 = d - 2*tr(A B) + ||A B||_F^2 expands into dot products
over per-kernel features, so the whole grid is one skinny matmul:
  X = W^T R,  W rows = [-2*vec(inv_i); vec(inv_i^T inv_i); d; 1],
              R rows = [vec(kn_j^T);  vec(kn_j kn_j^T);   1; m*_j]
The loss needs only the strict lower triangle (combined = tril(D)+triu(D^T)).
Each core computes a balanced subset of 128x512 lower-triangle tiles
("slots").  Masking of j >= i positions pushes X up by BIG so the hinge is 0:
  - column-only mask regions ride the extra rank-1 feature row (m*, staged
    per core — the program stays SPMD-uniform),
  - the 128x128 diagonal block is column-permuted to the tail of its chunk,
    where one shared strict-upper triangle constant is added via a second
    matmul (identity lhsT) — identical addresses on every core.
Per slot: PE matmul(s) -> ACT sqrt -> DVE min(s,1) with fused row-sum.
sum(relu(1-D)) = n_cols - sum(min(sqrt(X),1)) exactly, because inactive
entries contribute exact 1.0.  Host reduces in float64.
"""

import numpy as np
from contextlib import ExitStack

EPS = 1e-8
# Added to masked grid entries before sqrt.  Must dominate the matmul
# rounding error of the diagonal cancellation (can reach ~1e5 for
# near-singular kernels); 2^26 is exactly representable in fp32r.
BIG = float(2 ** 26)
N0, D0 = 2048, 3
N1, D1 = 1024, 5
F0 = 2 * D0 * D0 + 2   # 20: features + bias row + mask row
F1 = 2 * D1 * D1 + 2   # 52
NSLOT = 7              # 5 layer0 slots + 2 layer1 slots per core
N_CORES = 8

TRACE = False          # test harness sets True to capture an NTFF profile
LAST_RESULT = None     # BassKernelResults of the most recent run

_cached_nc = None


def _rtne12(x):
    """Round f32 to fp32r (11 explicit mantissa bits, round-nearest-even) —
    bit-exact match for what the PE consumes (verified on HW)."""
    u = np.ascontiguousarray(x, np.float32).view(np.uint32).astype(np.uint64)
    add = np.uint64(0x7FF) + ((u >> np.uint64(12)) & np.uint64(1))
    return ((u + add) >> np.uint64(12) << np.uint64(12)).astype(
        np.uint32).view(np.float32)


def _features(kernels, d):
    """Per-kernel features in float64: W (F, n), R (F, n).

    Last row of R (the mask row) is zero here; _stage_core fills it per slot.
    """
    k = np.asarray(kernels, np.float64)
    n = k.shape[0]
    norms = np.sqrt((k * k).sum(axis=(1, 2), keepdims=True)) + EPS
    kn = k / norms
    inv = np.linalg.inv(kn)
    G = np.einsum('iba,ibc->iac', inv, inv)        # inv^T @ inv
    H = np.einsum('iab,icb->iac', kn, kn)          # kn @ kn^T
    W = np.concatenate([
        -2.0 * inv.reshape(n, -1),
        G.reshape(n, -1),
        np.full((n, 1), float(d)),
        np.ones((n, 1)),
    ], axis=1).T
    R = np.concatenate([
        np.transpose(kn, (0, 2, 1)).reshape(n, -1),
        H.reshape(n, -1),
        np.ones((n, 1)),
        np.zeros((n, 1)),
    ], axis=1).T
    return W, R


def _diag_slot(W, R, t):
    """Build (w, r) for a diagonal slot of row tile t: columns permuted to
    [left | right | block] with the mask row set to BIG on the right region.
    The shared triangle constant later masks n' >= m inside the block."""
    F = W.shape[0]
    c = t // 4
    L = 128 * (t % 4)
    r = np.empty((F, 512), np.float64)
    r[:, :L] = R[:, 512 * c:512 * c + L]                    # left: keep
    r[:, L:384] = R[:, 128 * t + 128:512 * (c + 1)]         # right: mask
    r[:, 384:] = R[:, 128 * t:128 * t + 128]                # diagonal block
    r[F - 1, :] = 0.0
    r[F - 1, L:384] = BIG
    return W[:, 128 * t:128 * (t + 1)], r


def _full_slot(W, R, t, c):
    return (W[:, 128 * t:128 * (t + 1)],
            np.ascontiguousarray(R[:, 512 * c:512 * (c + 1)]))


def _core_slots(core, W0, R0, W1, R1):
    """7 slots: list of (w [F,128], r [F,512], has_corr).  Slots 0,1 are
    layer0 diagonal tiles, 2-4 layer0 full chunks, 5 layer1 diagonal,
    6 layer1 full chunk (cores >= 4) or fully-masked dummy (cores < 4)."""
    tA, tB = (core, 12 + core) if core < 4 else (core, core + 4)
    slots = [(*_diag_slot(W0, R0, t), True) for t in (tA, tB)]
    fulls = [(tA, c) for c in range(tA // 4)] + [(tB, c) for c in range(tB // 4)]
    assert len(fulls) == 3
    slots += [(*_full_slot(W0, R0, t, c), False) for (t, c) in fulls]
    t = core
    slots.append((*_diag_slot(W1, R1, t), True))
    w6, r6 = _full_slot(W1, R1, t, 0)
    if core < 4:
        r6 = r6.copy()
        r6[F1 - 1, :] = BIG                     # dummy: fully masked
    slots.append((w6, r6, False))
    return slots


def _split(a):
    """fp32r hi/lo split: a ~= hi + lo with both fp32r-exact."""
    hi = _rtne12(a.astype(np.float32))
    lo = _rtne12((a - hi.astype(np.float64)).astype(np.float32))
    return hi, lo


def _stage_core(core, W0, R0, W1, R1):
    """Per-core input map.

    Layer0 slots are error-compensated via one K=3*F0 matmul: wr0[s] is
    [3*F0, 640] staged as lhsT rows [Wh; Wl; Wh] and rhs rows [Rh; Rh; Rl],
    so X = Wh.Rh + Wl.Rh + Wh.Rl — fp32r speed, ~f32 accuracy (only the
    ~2^-24 Wl.Rl term is dropped).  Layer1 is single-pass fp32r [F1, 640]:
    it has no active pairs and a ~200x margin to the hinge threshold, so
    fp32r noise is harmless.
    """
    slots = _core_slots(core, W0, R0, W1, R1)
    wr0 = np.empty((5, 3 * F0, 640), np.float32)
    wr1 = np.empty((2, F1, 640), np.float32)
    for s, (w, r, _) in enumerate(slots):
        if s < 5:
            wh, wl = _split(w)
            rh, rl = _split(r)
            wr0[s, :F0, :128] = wh
            wr0[s, :F0, 128:] = rh
            wr0[s, F0:2 * F0, :128] = wl
            wr0[s, F0:2 * F0, 128:] = rh
            wr0[s, 2 * F0:, :128] = wh
            wr0[s, 2 * F0:, 128:] = rl
        else:
            wr1[s - 5, :, :128] = _rtne12(w.astype(np.float32))
            wr1[s - 5, :, 128:] = _rtne12(r.astype(np.float32))
    return {"wr0": wr0, "wr1": wr1}


def _build_program():
    import concourse.bacc as bacc
    import concourse.tile as tile
    import concourse.mybir as mybir

    f32 = mybir.dt.float32
    f32r = mybir.dt.float32r
    nc = bacc.Bacc("TRN2")
    wr0 = nc.declare_dram_parameter("wr0", [5, 3 * F0, 640], f32r,
                                    isOutput=False)
    wr1 = nc.declare_dram_parameter("wr1", [2, F1, 640], f32r, isOutput=False)
    acc = nc.declare_dram_parameter("acc", [NSLOT, 1], f32, isOutput=True)

    with tile.TileContext(nc) as tc, ExitStack() as ctx:
        cpool = ctx.enter_context(tc.tile_pool(name="const", bufs=1))
        ppool = ctx.enter_context(
            tc.tile_pool(name="psum", bufs=4, space="PSUM"))
        spool = ctx.enter_context(tc.tile_pool(name="work", bufs=3))

        # Shared constants first, so gpsimd's iotas aren't serialized behind
        # its DMA-queue drain: identity (corr lhsT) and the strict-upper
        # triangle TRI[k, n] = BIG * (n >= k), generated on-device.
        iota_c = cpool.tile([128, 128], f32)
        nc.gpsimd.iota(iota_c[:], pattern=[[1, 128]], base=0,
                       channel_multiplier=0,
                       allow_small_or_imprecise_dtypes=True)
        iota_p = cpool.tile([128, 1], f32)
        nc.gpsimd.iota(iota_p[:], pattern=[[1, 1]], base=0,
                       channel_multiplier=1,
                       allow_small_or_imprecise_dtypes=True)
        ones = cpool.tile([128, 1], f32)
        nc.gpsimd.memset(ones[:], 1.0)
        ident = cpool.tile([128, 128], f32r)
        nc.vector.tensor_scalar(
            out=ident[:], in0=iota_c[:], scalar1=iota_p[:, 0:1], scalar2=None,
            op0=mybir.AluOpType.is_equal)
        tri = cpool.tile([128, 128], f32r)
        nc.vector.tensor_scalar(
            out=tri[:], in0=iota_c[:], scalar1=iota_p[:, 0:1], scalar2=BIG,
            op0=mybir.AluOpType.is_ge, op1=mybir.AluOpType.mult)

        # Per-slot fused (w | r) tiles; separate DMAs, spread over engine
        # sequencers (each DMA_DIRECT2D occupies its issuer ~0.8-1us).  The
        # scalar engine loads the Sqrt table first (~1.3us), so it only
        # issues late slots.
        dma_engines = [nc.sync, nc.gpsimd, nc.sync, nc.gpsimd,
                       nc.sync, nc.scalar, nc.scalar]
        wr_tiles = []
        for s in range(NSLOT):
            F = 3 * F0 if s < 5 else F1
            src = wr0.ap()[s] if s < 5 else wr1.ap()[s - 5]
            t = cpool.tile([F, 640], f32r, tag=f"wr{s}")
            dma_engines[s].dma_start(t[:], src)
            wr_tiles.append(t)

        acc_t = cpool.tile([128, NSLOT], f32)
        has_corr = [True, True, False, False, False, True, False]
        for s in range(NSLOT):
            wr = wr_tiles[s]
            ps = ppool.tile([128, 512], f32)
            nc.tensor.matmul(ps[:], wr[:, 0:128], wr[:, 128:640],
                             start=True, stop=not has_corr[s])
            if has_corr[s]:
                nc.tensor.matmul(ps[:, 384:512], ident[:], tri[:],
                                 start=False, stop=True)
            sq = spool.tile([128, 512], f32, tag="sq")
            nc.scalar.activation(sq[:], ps[:],
                                 mybir.ActivationFunctionType.Sqrt)
            sc = spool.tile([128, 512], f32, tag="sc")
            nc.vector.tensor_scalar(
                out=sc[:], in0=sq[:], scalar1=1.0, scalar2=None,
                op0=mybir.AluOpType.min, op1=mybir.AluOpType.add,
                accum_out=acc_t[:, s:s + 1])
        # Partition-reduce acc on PE (out[s] = sum_p acc[p, s]) so the output
        # DMA is NSLOT packets instead of 128.
        # Subtract the 512-per-row baseline first: the remaining values are
        # exact zeros except rows with active pairs, so the f32 PE reduction
        # is exact to ~1e-7 (summing raw ~65536-magnitude rows costs ~1e-3).
        acc_d = cpool.tile([128, NSLOT], f32)
        nc.vector.tensor_scalar(
            out=acc_d[:], in0=acc_t[:], scalar1=512.0, scalar2=None,
            op0=mybir.AluOpType.subtract)
        acc_ps = ppool.tile([NSLOT, 1], f32, tag="accps")
        nc.tensor.matmul(acc_ps[:], acc_d[:], ones[:], start=True, stop=True)
        acc_sb = cpool.tile([NSLOT, 1], f32)
        nc.vector.tensor_copy(acc_sb[:], acc_ps[:])
        nc.sync.dma_start(acc.ap(), acc_sb[:])
    nc.compile()
    return nc


def _emulate_acc(in_map):
    """Numpy emulation of the device program (for sim/HW debugging)."""
    acc = np.zeros((128, NSLOT), np.float32)
    tri = (np.arange(128)[None, :] >= np.arange(128)[:, None]) * np.float32(BIG)
    has_corr = [True, True, False, False, False, True, False]
    for s in range(NSLOT):
        wr = in_map["wr0"][s] if s < 5 else in_map["wr1"][s - 5]
        x = wr[:, :128].T.astype(np.float32) @ wr[:, 128:]
        if has_corr[s]:
            x[:, 384:] += tri
        acc[:, s] = np.minimum(np.sqrt(x), 1.0).sum(axis=1)
    return (acc - np.float32(512.0)).sum(axis=0).reshape(NSLOT, 1)


def _reduce_loss(accs):
    """Host-side float64 reduction of per-core [NSLOT, 1] accumulators.

    Device returns sum_p (acc[p, s] - 512), so each slot's hinge total is
    just the negation.
    """
    S0 = 0.0
    S1 = 0.0
    for a in accs:
        a = np.asarray(a, np.float64).reshape(NSLOT)
        S0 += -a[:5].sum()
        S1 += -a[5:].sum()
    loss = 0.5 * (2.0 * S0 / (N0 * (N0 - 1)) + 2.0 * S1 / (N1 * (N1 - 1)))
    return np.float32(loss)


def _get_program():
    global _cached_nc
    if _cached_nc is None:
        _cached_nc = _build_program()
    return _cached_nc


def kernel(kernels0, kernels1):
    global LAST_RESULT
    from concourse.bass_utils import run_bass_kernel_spmd

    W0, R0 = _features(kernels0, D0)
    W1, R1 = _features(kernels1, D1)
    in_maps = [_stage_core(c, W0, R0, W1, R1) for c in range(N_CORES)]
    nc = _get_program()
    res = run_bass_kernel_spmd(nc, in_maps, list(range(N_CORES)), trace=TRACE)
    LAST_RESULT = res
    accs = [res.results[c]["acc"] for c in range(N_CORES)]
    return _reduce_loss(accs)
